# revision 41
# baseline (speedup 1.0000x reference)
"""Trainium2 Bass kernel for nn_Attention (dense transformer attention block).

Reference computation (per batch b):
  q = BN(wq @ x) -> (8 heads, 16, 3136);  k likewise;  v = BN(wv @ x) -> (8, 64, 3136)
  attn = softmax(q^T k) over 3136x3136 tokens (no 1/sqrt(d) scaling)
  o = attn @ v^T -> (512, 56, 56);  out = BN(wp @ o) -> (256, 56, 56)

Sharding: 8 cores = 2 batches x 4 query-token chunks of 784. Each core
computes k/v for all 3136 key tokens (cheap, redundant) and attention +
output projection for its own 784 query tokens. Zero collectives; the host
assembles the 8 output shards.

Device algorithm per core (flash-style; bf16 matmuls, f32 PSUM accumulation):
  - Softmax exp splits across ACT (exact exp, activation-scale undoes the
    A16=128/ln2 factor folded into wq/bq host-side) and DVE (one-op
    Schraudolph bit-trick: int16(s + b) bit-cast to bf16 ~= exp(s/A16)*2^-7;
    the 2^-7 cancels in the softmax divide).
  - k-bias dropped (softmax-invariant); v-bias folded into the output
    projection bias host-side.
  - Scores S_T[m, n] = k_blk^T q run as K=32 matmuls in 4 rotating 32-row
    PE tiles (16 data rows + 16 zero rows; K=16 without the zero-fill works
    but leaves stale weights toggling in the PE array, which measurably
    increases HAM power throttling and is a net loss).
  - o'[65, n] += v'^T_blk @ p_blk; row 64 (ones column) accumulates the
    softmax denominator. o' matmuls of iteration i run as PE "filler"
    inside iteration i+1's scores/exp phase in four batches (fewer PE
    tile-config switches, batches after groups 2/4/6); the v'^T projection
    runs as iteration-0 filler.
  - Inputs load as ~100-200KB DMA chunks spread across the sync/scalar/
    gpsimd queues so they stripe over many DMA rings.
  - softmax divide: o' PSUM evacuates promptly (ACT) so the banks recycle
    fast; den row -> partitions 0-1 via SBUF->SBUF DMA -> DVE reciprocal ->
    DRAM bounce for the 64-partition broadcast -> multiply on GpSimd (keeps
    the DVE queue clear). All divide-chain DMAs ride the quiet gpsimd
    queue, not sync.
  - out = wp_eff @ o; chunk 0's projection runs as iteration-5 filler,
    chunk 1's accumulates in the (by then free) scores PSUM banks, with the
    head contraction ordered to match flush completion order.

Measured on 8xTRN2 cores: ~212-222 us depending on HAM/power throttle state
(baseline of this session: 218-259 us on the same spread); rel err 2.3e-3.
DVE/ACT busy reduced vs baseline by ~32/21 us, which helps most when the
clocks throttle.
"""

import math
import os
import sys

for _p in ("/opt/trn_rl_repo", "/root/.axon_site/_ro/trn_rl_repo"):
    if os.path.isdir(_p) and _p not in sys.path:
        sys.path.insert(0, _p)

import numpy as np

NUM_HEADS = 8
KEY_DIM = 16
D_HEAD = 64
B = 2
C = 256
HH = 56
WW = 56
N = HH * WW          # 3136 tokens
NCHUNK = N // 4      # 784 query tokens per core
NSUB = NCHUNK // 2   # 392, fits one PSUM bank
NB = (N + 127) // 128            # 25 key-blocks
MB_SIZES = [128] * 24 + [64]
KS = [128, 128]                  # contraction chunks for K=256
GROUPS = [list(range(g * 3, min(g * 3 + 3, NB))) for g in range(9)]
ACT_E1_GROUPS = frozenset((4, 8))  # e1 exp groups on ACT (rest DVE); keeps
# ACT (~112us busy) and DVE (~130us) balanced so scores don't stall on the
# DVE queue late in each iteration

A16 = 128.0 / math.log(2.0)          # scale folded into wq: scores = A16*S
B_SCH = 16256.0 - 896.0 - 7.0        # Schraudolph bias incl. 2^-7 and c=7
ACT_SCALE = math.log(2.0) / 128.0    # undoes A16 on the exact-exp path
LN2_7 = -7.0 * math.log(2.0)         # exp bias; cancels in the divide

_GRAPH = None
NUM_DEVICES = 8
DEBUG = False


def _build_graph():
    import concourse.bass as bass  # noqa: F401
    import concourse.mybir as mybir
    import concourse.tile as tile
    from concourse import bacc
    from contextlib import ExitStack

    f32 = mybir.dt.float32
    bf16 = mybir.dt.bfloat16
    i16 = mybir.dt.int16
    Exp = mybir.ActivationFunctionType.Exp

    nc = bacc.Bacc("TRN2", target_bir_lowering=False, debug=False,
                   num_devices=NUM_DEVICES)
    xa_d = nc.dram_tensor("xa", [256, N], bf16, kind="ExternalInput").ap()
    xq_d = nc.dram_tensor("xq", [256, NCHUNK], bf16, kind="ExternalInput").ap()
    wq_d = nc.dram_tensor("wq", [256, 128], bf16, kind="ExternalInput").ap()
    wk_d = nc.dram_tensor("wk", [256, 128], bf16, kind="ExternalInput").ap()
    wv_d = nc.dram_tensor("wv", [256, 512], bf16, kind="ExternalInput").ap()
    qb_d = nc.dram_tensor("qb", [128, 1], f32, kind="ExternalInput").ap()
    pb_d = nc.dram_tensor("pb", [128, 2], f32, kind="ExternalInput").ap()
    wp_d = nc.dram_tensor("wp", [64, 8, 256], bf16, kind="ExternalInput").ap()
    out_d = nc.dram_tensor("out", [256, NCHUNK], f32, kind="ExternalOutput").ap()
    if DEBUG:
        rsr_d = nc.dram_tensor("rsr", [16, NSUB], f32,
                               kind="ExternalOutput").ap()
    else:
        rsr_d = nc.dram_tensor("rsr", [16, NSUB], f32).ap()
    if DEBUG:
        dbg = {nm: nc.dram_tensor("dbg_" + nm, shp, dt,
                                  kind="ExternalOutput").ap()
               for nm, shp, dt in [
                   ("qsb", [128, NCHUNK], bf16), ("ksb", [128, N], bf16),
                   ("klo", [128, N], bf16), ("khi2", [128, N], bf16),
                   ("vt", [128, NB * 2 * 4 * 65], bf16),
                   ("of", [64, 8 * NCHUNK], bf16)]}

    with tile.TileContext(nc) as tc, ExitStack() as stk:
        const = stk.enter_context(tc.tile_pool(name="const", bufs=1))
        xq_sb = const.tile([128, 2, NCHUNK], bf16, tag="xq")
        wq_sb = const.tile([128, 2, 128], bf16, tag="wq")
        wk_sb = const.tile([128, 2, 128], bf16, tag="wk")
        wv_sb = const.tile([128, 2, 512], bf16, tag="wv")
        wp_sb = const.tile([64, 8, 256], bf16, tag="wp")
        qb_sb = const.tile([128, 1], f32, tag="qb")
        pb_sb = const.tile([128, 2], f32, tag="pb")
        eb_sb = const.tile([128, 1], f32, tag="eb")
        k_lo = const.tile([128, N], bf16, tag="klo")
        k_hi = const.tile([128, N], bf16, tag="khi")
        q_lo = const.tile([128, NCHUNK], bf16, tag="qlo")
        q_hi = const.tile([128, NCHUNK], bf16, tag="qhi")
        # replicas shifted by +32 partitions so consecutive blocks of one head
        # use different PE row tiles
        k_lo2 = const.tile([128, N], bf16, tag="klo2")
        k_hi2 = const.tile([128, N], bf16, tag="khi2")
        q_lo2 = const.tile([128, NCHUNK], bf16, tag="qlo2")
        q_hi2 = const.tile([128, NCHUNK], bf16, tag="qhi2")
        # v'^T: [m-in-block, block, half, head-in-half, 64 v cols + ones col]
        vT_sb = const.tile([128, NB, 2, 4, 65], bf16, tag="vt")
        of_sb = const.tile([64, 8, NCHUNK], bf16, tag="of")
        y_sb = const.tile([128, 2, NCHUNK], f32, tag="y")
        xa_sb = const.tile([128, 2, N], bf16, tag="xa")
        k_sb = const.tile([128, N], bf16, tag="ksb")
        q_sb = const.tile([128, NCHUNK], bf16, tag="qsb")

        # ---- input DMAs, chunked across sync/gpsimd/scalar HWDGE queues so
        # the big loads stripe over many DMA rings instead of serializing.
        # xa n-quarter j feeds k-proj pair j and v' blocks ~6j..6j+6, so xa
        # chunks lead each queue in j order; weights slot between them ----
        NA8 = N // 8

        def xa_chunk(eng, j, kc):
            eng.dma_start(
                out=xa_sb[:, kc, j * NA8:(j + 1) * NA8],
                in_=xa_d[128 * kc:128 * kc + 128, j * NA8:(j + 1) * NA8])

        def xq_chunk(eng, j, kc):
            eng.dma_start(
                out=xq_sb[:, kc, j * NSUB:(j + 1) * NSUB],
                in_=xq_d[128 * kc:128 * kc + 128, j * NSUB:(j + 1) * NSUB])

        # n-chunk j of xa feeds k-proj chunk ~j/2 and v' blocks ~3j..3j+3;
        # the first four chunks land on four separate rings so the k/v
        # projections start ~6us in
        xa_chunk(nc.sync, 0, 0)
        xa_chunk(nc.scalar, 0, 1)
        xa_chunk(nc.gpsimd, 1, 0)
        xa_chunk(nc.gpsimd, 1, 1)
        nc.sync.dma_start(out=wq_sb[:, 0, :], in_=wq_d[0:128, :])
        nc.sync.dma_start(out=wq_sb[:, 1, :], in_=wq_d[128:256, :])
        nc.sync.dma_start(out=qb_sb[:], in_=qb_d)
        nc.scalar.dma_start(out=wv_sb[:, 0, :], in_=wv_d[0:128, :])
        nc.scalar.dma_start(out=wv_sb[:, 1, :], in_=wv_d[128:256, :])
        nc.gpsimd.dma_start(out=wk_sb[:, 0, :], in_=wk_d[0:128, :])
        nc.gpsimd.dma_start(out=wk_sb[:, 1, :], in_=wk_d[128:256, :])
        xa_chunk(nc.sync, 2, 0)
        xa_chunk(nc.scalar, 2, 1)
        xq_chunk(nc.gpsimd, 0, 0)
        xq_chunk(nc.gpsimd, 0, 1)
        xa_chunk(nc.sync, 3, 0)
        xa_chunk(nc.scalar, 3, 1)
        xa_chunk(nc.gpsimd, 4, 0)
        xa_chunk(nc.gpsimd, 4, 1)
        xq_chunk(nc.sync, 1, 0)
        xq_chunk(nc.scalar, 1, 1)
        xa_chunk(nc.sync, 5, 0)
        xa_chunk(nc.scalar, 5, 1)
        xa_chunk(nc.gpsimd, 6, 0)
        xa_chunk(nc.gpsimd, 6, 1)
        xa_chunk(nc.sync, 7, 0)
        xa_chunk(nc.scalar, 7, 1)
        nc.sync.dma_start(out=pb_sb[:], in_=pb_d)
        nc.sync.dma_start(out=wp_sb[:, 0:4, :], in_=wp_d[:, 0:4, :])
        nc.sync.dma_start(out=wp_sb[:, 4:8, :], in_=wp_d[:, 4:8, :])
        nc.vector.memset(eb_sb[:], LN2_7)
        nc.vector.memset(vT_sb[:, :, :, :, 64:65], 1.0)
        # zero-fill: K=32 score matmuls contract 16 zero rows per tile so the
        # PE array's unused rows hold zero weights (K=16 would leave stale
        # weights toggling against real data -> measurably more HAM throttle)
        for t in (k_lo, k_hi, k_lo2, k_hi2, q_lo, q_hi, q_lo2, q_hi2):
            nc.vector.memset(t[:], 0.0)

        with tc.tile_pool(name="pP", bufs=30) as pP, \
             tc.tile_pool(name="pEp", bufs=4) as pEp, \
             tc.tile_pool(name="psO", bufs=2, space="PSUM") as psO, \
             tc.tile_pool(name="psS", bufs=2, space="PSUM") as psS:

            # ---- projections: q then k (PE), kc-chunks interleaved across
            # bank pairs to hide the PSUM read-modify-write stall ----
            q_ps2 = [psO.tile([128, 512], f32, tag="ops", name=f"qps{c2}")
                     for c2 in range(2)]
            for kc in range(2):
                for c2 in range(2):
                    nc.tensor.matmul(
                        q_ps2[c2][0:128, 0:NSUB],
                        wq_sb[0:KS[kc], kc, :],
                        xq_sb[0:KS[kc], kc, c2 * NSUB:(c2 + 1) * NSUB],
                        start=(kc == 0), stop=(kc == 1))
            for c2 in range(2):
                nc.scalar.add(
                    q_sb[:, c2 * NSUB:(c2 + 1) * NSUB],
                    q_ps2[c2][0:128, 0:NSUB], qb_sb[:, 0:1])
            KP = [(512 * p, min(512, N - 512 * p)) for p in range(7)]
            for p0 in range(0, 7, 2):
                ps = [(p, psO.tile([128, 512], f32, tag="ops", name=f"kps{p}"))
                      for p in range(p0, min(p0 + 2, 7))]
                for kc in range(2):
                    for p, k_ps in ps:
                        c0, cw = KP[p]
                        nc.tensor.matmul(
                            k_ps[0:128, 0:cw],
                            wk_sb[0:KS[kc], kc, :],
                            xa_sb[0:KS[kc], kc, c0:c0 + cw],
                            start=(kc == 0), stop=(kc == 1))
                for p, k_ps in ps:
                    c0, cw = KP[p]
                    if p % 2 == 0:
                        nc.vector.tensor_copy(k_sb[:, c0:c0 + cw],
                                              k_ps[0:128, 0:cw])
                    else:
                        nc.scalar.copy(k_sb[:, c0:c0 + cw], k_ps[0:128, 0:cw])

            # ---- q/k regroup: heads are already at 32-aligned slots in
            # q_sb/k_sb (host-side weight permutation), so each of q_lo /
            # q_hi / replicas fills with one strided DMA (two for the
            # wrapped +32 replica) ----
            def regroup(dst_lo, dst_hi, dst_lo2, dst_hi2, src, n, engs):
                for h in range(8):
                    dt_ = dst_lo if h < 4 else dst_hi
                    dt2 = dst_lo2 if h < 4 else dst_hi2
                    bp_ = 32 * (h % 4)
                    bp2 = (bp_ + 32) % 128
                    eng = engs[h % len(engs)]
                    eng.dma_start(out=dt_[bp_:bp_ + 16, 0:n],
                                  in_=src[16 * h:16 * h + 16, 0:n])
                    eng.dma_start(out=dt2[bp2:bp2 + 16, 0:n],
                                  in_=src[16 * h:16 * h + 16, 0:n])

            regroup(q_lo, q_hi, q_lo2, q_hi2, q_sb, NCHUNK, [nc.gpsimd])
            regroup(k_lo, k_hi, k_lo2, k_hi2, k_sb, N, [nc.sync, nc.gpsimd])

            # ---- main attention loop, software-pipelined ----
            # iteration i = (head-pair, n-chunk). During iteration i's
            # scores+exp phase the PE runs iteration i-1's o'-accumulation
            # matmuls as filler (iterations 0/1 run the v'^T projection, and
            # iteration 5 additionally chunk 0's output projection).
            # PSUM: scores 2 x 3 banks + o'/v'/wp pool 2 x 1 bank = 8 banks.
            PAIRS = [(0, 2), (1, 3), (4, 6), (5, 7)]
            ITERS = [(pair, c2) for c2 in range(2) for pair in PAIRS]

            def make_o_filler(pair, e, p_tile, i, mb, o_ps2):
                def emit():
                    h = pair[e]
                    pbi = MB_SIZES[mb]
                    nc.tensor.matmul(
                        o_ps2[e][0:65, 0:NSUB],
                        vT_sb[0:pbi, mb, h // 4, h % 4, :],
                        p_tile[0:pbi, i, 0:NSUB],
                        start=(mb == 0), stop=(mb == NB - 1))
                return emit

            def make_v_filler(mb):
                def emit():
                    pb_ = MB_SIZES[mb]
                    vt_ps = psO.tile([128, 2, 4, 64], f32, tag="ops",
                                     name=f"vtps{mb}")
                    for kc in range(2):
                        nc.tensor.matmul(
                            vt_ps[0:pb_, :, :, :],
                            xa_sb[0:KS[kc], kc, mb * 128:mb * 128 + pb_],
                            wv_sb[0:KS[kc], kc, :],
                            start=(kc == 0), stop=(kc == 1))
                    if mb % 2 == 0:
                        nc.vector.tensor_copy(
                            vT_sb[0:pb_, mb, :, :, 0:64], vt_ps[0:pb_, :, :, :])
                    else:
                        nc.scalar.copy(
                            vT_sb[0:pb_, mb, :, :, 0:64], vt_ps[0:pb_, :, :, :])
                return emit

            def make_wp_jobs(c2):
                # contract heads in pipeline-completion order: the last
                # pair's heads (5, 7) come last so earlier matmuls run while
                # the final epilogue's divide chain is still in flight
                nc0 = c2 * NSUB
                KC_ORDER = (0, 2, 1, 3, 4, 6, 5, 7)

                def job(mo, pool, tag):
                    def emit():
                        y_ps = pool.tile([128, 512], f32, tag=tag,
                                         name=f"yps{c2}{mo}",
                                         padded_shape=None)
                        for j, kc in enumerate(KC_ORDER):
                            nc.tensor.matmul(
                                y_ps[0:128, 0:NSUB],
                                wp_sb[0:64, kc, mo * 128:(mo + 1) * 128],
                                of_sb[0:64, kc, nc0:nc0 + NSUB],
                                start=(j == 0), stop=(j == 7))
                        nc.vector.tensor_scalar_add(
                            y_sb[:, mo, nc0:nc0 + NSUB], y_ps[0:128, 0:NSUB],
                            pb_sb[:, mo:mo + 1])
                        nc.sync.dma_start(
                            out=out_d[mo * 128:(mo + 1) * 128,
                                      nc0:nc0 + NSUB],
                            in_=y_sb[:, mo, nc0:nc0 + NSUB])
                    return emit
                if c2 == 0:
                    return [job(0, psO, "ops"), job(1, psO, "ops")]
                return [job(0, psS, "sps"), job(1, psS, "sps")]

            def divide_chain(pair, c2, o_ps2, it):
                # evacuate o' PSUM promptly (frees the banks for the next
                # iteration's fillers; the divide chain below can lag); den
                # row moves to partitions 0-1 via SBUF->SBUF DMA for the
                # custom-DVE reciprocal, then DRAM bounce for the broadcast.
                # Chain DMAs ride the quiet gpsimd queue, not sync.
                nc0 = c2 * NSUB
                rsh = pEp.tile([66, 2, NSUB], f32, tag="rsh", bufs=2,
                               name=f"rsh{it}")
                nc.scalar.copy(rsh[64:65, 0, :], o_ps2[0][64:65, 0:NSUB])
                nc.vector.tensor_copy(rsh[64:65, 1, :], o_ps2[1][64:65, 0:NSUB])
                ous = []
                for e in range(2):
                    ou = pEp.tile([64, NSUB], f32, tag=f"ou{e}",
                                  name=f"ou{it}{e}", bufs=2)
                    nc.scalar.copy(ou[0:64, :], o_ps2[e][0:64, 0:NSUB])
                    ous.append(ou)
                rb2 = pEp.tile([2, NSUB], f32, tag="rb2", bufs=2,
                               name=f"rb2{it}")
                nc.gpsimd.dma_start(out=rb2[0:2, :], in_=rsh[64:65, :, :])
                rbr = pEp.tile([2, NSUB], f32, tag="rbr", bufs=2,
                               name=f"rbr{it}")
                scr = pEp.tile([2, NSUB], f32, tag="scr", bufs=2,
                               name=f"scr{it}")
                nc.vector.reciprocal_approx_accurate(
                    out=rbr[0:2, :], in_=rb2[0:2, :], scratch=scr[0:2, :])
                nc.gpsimd.dma_start(out=rsr_d[2 * it:2 * it + 2, :],
                                    in_=rbr[0:2, :])
                for e in range(2):
                    h = pair[e]
                    rbc = pEp.tile([64, NSUB], f32, tag=f"rbc{e}", bufs=2,
                                   name=f"rbc{it}{e}")
                    nc.gpsimd.dma_start(
                        out=rbc[0:64, :],
                        in_=rsr_d[2 * it + e:2 * it + e + 1,
                                  :].partition_broadcast(64))
                    # multiply on Pool (SBUF-only operands): keeps the DVE
                    # queue clear of mid-iteration latency spikes
                    nc.gpsimd.tensor_mul(
                        out=of_sb[0:64, h, nc0:nc0 + NSUB],
                        in0=ous[e][0:64, :], in1=rbc[0:64, :])

            prev = None  # (pair, c2, p_tiles, it) of the previous iteration
            for it in range(len(ITERS) + 1):
                cur = ITERS[it] if it < len(ITERS) else None
                fillers = []
                if it == 0:
                    # v' blocks 0-14 run immediately (they only need xa/wv,
                    # ready early) while the first scores wait on the q/k
                    # regroup; 15-24 interleave with iteration 0's groups
                    for mb in range(15):
                        make_v_filler(mb)()
                    fillers = [make_v_filler(mb) for mb in range(15, NB)]
                if prev is not None:
                    ppair, pc2, p_tiles, pit = prev
                    o_ps2 = [psO.tile([128, 512], f32, tag="ops",
                                      name=f"ops{e}") for e in range(2)]
                    for g2, blocks2 in enumerate(GROUPS):
                        for i2, mb2 in enumerate(blocks2):
                            for e in range(2):
                                fillers.append(make_o_filler(
                                    ppair, e, p_tiles[g2][e], i2, mb2, o_ps2))
                if it == 5:
                    fillers.extend(make_wp_jobs(0))
                if cur is None:
                    # flush: run each head's o' fillers then its divide chain
                    # immediately, so head 0's chain hides under head 1's 25
                    # filler matmuls
                    nc0f = pc2 * NSUB
                    for e in range(2):
                        for job in fillers[e::2]:
                            job()
                        # flush: shortest chain — den out, 64-row broadcast,
                        # reciprocal at partitions 0-63, multiply straight
                        # from the o' PSUM (no bank reuse pressure at flush)
                        h = ppair[e]
                        rshF = pEp.tile([66, 2, NSUB], f32, tag="rsh",
                                        bufs=2, name=f"rshF{e}")
                        if e == 0:
                            nc.scalar.copy(rshF[64:65, 0, :],
                                           o_ps2[e][64:65, 0:NSUB])
                        else:
                            nc.vector.tensor_copy(rshF[64:65, 0, :],
                                                  o_ps2[e][64:65, 0:NSUB])
                        nc.gpsimd.dma_start(
                            out=rsr_d[2 * pit + e:2 * pit + e + 1, :],
                            in_=rshF[64:65, 0, :])
                        rbcF = pEp.tile([64, NSUB], f32, tag=f"rbc{e}",
                                        bufs=2, name=f"rbcF{e}")
                        nc.gpsimd.dma_start(
                            out=rbcF[0:64, :],
                            in_=rsr_d[2 * pit + e:2 * pit + e + 1,
                                      :].partition_broadcast(64))
                        rbrF = pEp.tile([64, NSUB], f32, tag=f"rbrF{e}",
                                        bufs=1, name=f"rbrF{e}")
                        scrF = pEp.tile([64, NSUB], f32, tag=f"scrF{e}",
                                        bufs=1, name=f"scrF{e}")
                        nc.vector.reciprocal_approx_accurate(
                            out=rbrF[0:64, :], in_=rbcF[0:64, :],
                            scratch=scrF[0:64, :])
                        nc.vector.tensor_mul(
                            out=of_sb[0:64, h, nc0f:nc0f + NSUB],
                            in0=o_ps2[e][0:64, 0:NSUB], in1=rbrF[0:64, :])
                    for job in make_wp_jobs(1):
                        job()
                    if DEBUG:
                        nc.sync.dma_start(out=dbg["qsb"], in_=q_sb[:])
                        nc.sync.dma_start(out=dbg["ksb"], in_=k_sb[:])
                        nc.sync.dma_start(out=dbg["klo"], in_=k_lo[:])
                        nc.sync.dma_start(out=dbg["khi2"], in_=k_hi2[:])
                        nc.sync.dma_start(out=dbg["vt"], in_=vT_sb[:])
                        nc.sync.dma_start(out=dbg["of"], in_=of_sb[:])
                    break
                pair, c2 = cur
                kts = [k_lo if h < 4 else k_hi for h in pair]
                qts = [q_lo if h < 4 else q_hi for h in pair]
                kts2 = [k_lo2 if h < 4 else k_hi2 for h in pair]
                qts2 = [q_lo2 if h < 4 else q_hi2 for h in pair]
                bps = [32 * (h % 4) for h in pair]
                bps2 = [(32 * (h % 4) + 32) % 128 for h in pair]
                p_tiles = []
                nfill = len(fillers)
                nc0 = c2 * NSUB
                for g, blocks in enumerate(GROUPS):
                    gsz = len(blocks)
                    pb_ = MB_SIZES[blocks[-1]]
                    # two decoupled (PSUM-buffer, exp-engine) chains; score
                    # matmuls interleave (mb, e) across 4 rotating K=16 row
                    # tiles for PE stream concurrency
                    s_ps2 = [psS.tile([128, 3, 512], f32, tag="sps",
                                      name=f"sps{e}") for e in range(2)]
                    for i, mb in enumerate(blocks):
                        pbi = MB_SIZES[mb]
                        for e in range(2):
                            if mb % 2 == 0:
                                kte, qte, be = kts[e], qts[e], bps[e]
                            else:
                                kte, qte, be = kts2[e], qts2[e], bps2[e]
                            nc.tensor.matmul(
                                s_ps2[e][0:pbi, i, 0:NSUB],
                                kte[be:be + 32, mb * 128:mb * 128 + pbi],
                                qte[be:be + 32, nc0:nc0 + NSUB],
                                start=True, stop=True,
                                tile_position=(be, 0))
                    p_sb2 = []
                    for e in range(2):
                        p_sb = pP.tile([128, 3, NSUB], bf16, tag="psb",
                                       name=f"psb{e}")
                        if e == 0 or g in ACT_E1_GROUPS:
                            nc.scalar.activation(
                                out=p_sb[0:pb_, 0:gsz, 0:NSUB],
                                in_=s_ps2[e][0:pb_, 0:gsz, 0:NSUB], func=Exp,
                                bias=eb_sb[0:pb_, 0:1], scale=ACT_SCALE)
                        else:
                            nc.vector.tensor_scalar_add(
                                p_sb[0:pb_, 0:gsz, 0:NSUB].bitcast(i16),
                                s_ps2[e][0:pb_, 0:gsz, 0:NSUB], B_SCH)
                        p_sb2.append(p_sb)
                    p_tiles.append(p_sb2)
                    # interleave previous iteration's o' matmuls as PE
                    # filler in three big batches (after groups 2/4/6) —
                    # fewer tile-config switches than per-group batches; the
                    # epilogue at slot 6 still frees the o' banks in time
                    FS = {2: (0, 1), 4: (1, 2), 6: (2, 3)}
                    if g in FS:
                        lo = nfill * FS[g][0] // 3
                        hi = nfill * FS[g][1] // 3
                        for job in fillers[lo:hi]:
                            job()
                    if g == 6 and prev is not None:
                        divide_chain(ppair, pc2, o_ps2, pit)
                prev = (pair, c2, p_tiles, it)

    nc.compile()
    return nc


def get_graph():
    global _GRAPH
    if _GRAPH is None:
        _GRAPH = _build_graph()
    return _GRAPH


def make_in_maps(x, wq, sq, bq, wk, sk, bk, wv, sv, bv, wp, sp, bp):
    import ml_dtypes
    bf = ml_dtypes.bfloat16
    f = np.float32
    x2 = np.asarray(x, f).reshape(B, C, N)
    wq = np.asarray(wq, f); sq = np.asarray(sq, f); bq = np.asarray(bq, f)
    wk = np.asarray(wk, f); sk = np.asarray(sk, f)
    wv = np.asarray(wv, f); sv = np.asarray(sv, f); bv = np.asarray(bv, f)
    wp = np.asarray(wp, f); sp = np.asarray(sp, f); bp = np.asarray(bp, f)

    wq_eff = ((wq * sq[:, None]).T * A16).astype(f)   # (256, 128), pre-scaled
    wk_eff = (wk * sk[:, None]).T.astype(f)           # k-bias dropped
    qb_eff = (bq * A16).astype(f)
    wv_base = wv * sv[:, None]  # (512, 256)
    wv_arr = np.zeros((256, 512), f)
    for h in range(NUM_HEADS):
        col = 256 * (h // 4) + 64 * (h % 4)
        wv_arr[:, col:col + 64] = wv_base[64 * h:64 * h + 64, :].T
    wp_sc = wp * sp[:, None]                 # (256, 512)
    wp_eff = wp_sc.T.astype(f)               # (512, 256), row c = 64h+d
    wp_arr = wp_eff.reshape(8, 64, 256).transpose(1, 0, 2).copy()
    pb_fold = (bp + wp_sc @ bv).astype(f)    # v-bias folded into out-bias
    pb_arr = pb_fold.reshape(2, 128).T.copy()  # pb_arr[d, mo] = pb'[128*mo+d]
    in_maps = []
    for core in range(8):
        b, j = core // 4, core % 4
        xa_full = np.ascontiguousarray(x2[b])
        xq_c = np.ascontiguousarray(xa_full[:, j * NCHUNK:(j + 1) * NCHUNK])
        in_maps.append(dict(
            xa=xa_full.astype(bf), xq=xq_c.astype(bf),
            wq=wq_eff.astype(bf), wk=wk_eff.astype(bf),
            wv=wv_arr.astype(bf), wp=wp_arr.astype(bf),
            qb=qb_eff.reshape(128, 1).astype(f),
            pb=pb_arr.astype(f)))
    return in_maps


def assemble_output(results):
    y = np.zeros((B, C, N), np.float32)
    for core in range(8):
        b, j = core // 4, core % 4
        y[b, :, j * NCHUNK:(j + 1) * NCHUNK] = results[core]["out"]
    return y.reshape(B, C, HH, WW)


def kernel(**inputs):
    from concourse.bass_utils import run_bass_kernel_spmd
    nc = get_graph()
    in_maps = make_in_maps(**inputs)
    res = run_bass_kernel_spmd(nc, in_maps, core_ids=list(range(8)))
    return assemble_output(res.results)


if __name__ == "__main__":
    rng = np.random.default_rng(0)
    ins = dict(
        x=rng.standard_normal((2, 256, 56, 56), np.float32).astype(np.float32),
        wq=(rng.standard_normal((128, 256)) * 0.05).astype(np.float32),
        sq=rng.random(128).astype(np.float32),
        bq=(rng.standard_normal(128) * 0.05).astype(np.float32),
        wk=(rng.standard_normal((128, 256)) * 0.05).astype(np.float32),
        sk=rng.random(128).astype(np.float32),
        bk=(rng.standard_normal(128) * 0.05).astype(np.float32),
        wv=(rng.standard_normal((512, 256)) * 0.05).astype(np.float32),
        sv=rng.random(512).astype(np.float32),
        bv=(rng.standard_normal(512) * 0.05).astype(np.float32),
        wp=(rng.standard_normal((256, 512)) * 0.05).astype(np.float32),
        sp=rng.random(256).astype(np.float32),
        bp=(rng.standard_normal(256) * 0.05).astype(np.float32),
    )
    out = kernel(**ins)
    print("out", out.shape, out.dtype, float(np.abs(out).mean()))


# revision 44
# speedup vs baseline: 1.0030x; 1.0030x over previous
"""Trainium2 Bass kernel for nn_Attention (dense transformer attention block).

Reference computation (per batch b):
  q = BN(wq @ x) -> (8 heads, 16, 3136);  k likewise;  v = BN(wv @ x) -> (8, 64, 3136)
  attn = softmax(q^T k) over 3136x3136 tokens (no 1/sqrt(d) scaling)
  o = attn @ v^T -> (512, 56, 56);  out = BN(wp @ o) -> (256, 56, 56)

Sharding: 8 cores = 2 batches x 4 query-token chunks of 784. Each core
computes k/v for all 3136 key tokens (cheap, redundant) and attention +
output projection for its own 784 query tokens. Zero collectives; the host
assembles the 8 output shards.

Device algorithm per core (flash-style; bf16 matmuls, f32 PSUM accumulation):
  - Softmax exp splits across ACT (exact exp, activation-scale undoes the
    A16=128/ln2 factor folded into wq/bq host-side) and DVE (one-op
    Schraudolph bit-trick: int16(s + b) bit-cast to bf16 ~= exp(s/A16)*2^-7;
    the 2^-7 cancels in the softmax divide).
  - k-bias dropped (softmax-invariant); v-bias folded into the output
    projection bias host-side.
  - Scores S_T[m, n] = k_blk^T q run as K=32 matmuls in 4 rotating 32-row
    PE tiles (16 data rows + 16 zero rows; K=16 without the zero-fill works
    but leaves stale weights toggling in the PE array, which measurably
    increases HAM power throttling and is a net loss).
  - o'[65, n] += v'^T_blk @ p_blk; row 64 (ones column) accumulates the
    softmax denominator. o' matmuls of iteration i run as PE "filler"
    inside iteration i+1's scores/exp phase in four batches (fewer PE
    tile-config switches, batches after groups 2/4/6); the v'^T projection
    runs as iteration-0 filler.
  - Inputs load as ~100-200KB DMA chunks spread across the sync/scalar/
    gpsimd queues so they stripe over many DMA rings.
  - softmax divide: o' PSUM evacuates promptly (ACT) so the banks recycle
    fast; den row -> partitions 0-1 via SBUF->SBUF DMA -> DVE reciprocal ->
    DRAM bounce for the 64-partition broadcast -> multiply on GpSimd (keeps
    the DVE queue clear). All divide-chain DMAs ride the quiet gpsimd
    queue, not sync.
  - out = wp_eff @ o; chunk 0's projection runs as iteration-5 filler,
    chunk 1's accumulates in the (by then free) scores PSUM banks. The
    flush runs head e1's o' fillers and divide chain first (the chain
    hides under e0's fillers) and chunk 1's wp contraction ends on e0's
    head, minimizing the exposed tail.

Measured on 8xTRN2 cores: ~212-222 us depending on HAM/power throttle state
(baseline of this session: 218-259 us on the same spread); rel err 2.3e-3.
DVE/ACT busy reduced vs baseline by ~32/21 us, which helps most when the
clocks throttle.
"""

import math
import os
import sys

for _p in ("/opt/trn_rl_repo", "/root/.axon_site/_ro/trn_rl_repo"):
    if os.path.isdir(_p) and _p not in sys.path:
        sys.path.insert(0, _p)

import numpy as np

NUM_HEADS = 8
KEY_DIM = 16
D_HEAD = 64
B = 2
C = 256
HH = 56
WW = 56
N = HH * WW          # 3136 tokens
NCHUNK = N // 4      # 784 query tokens per core
NSUB = NCHUNK // 2   # 392, fits one PSUM bank
NB = (N + 127) // 128            # 25 key-blocks
MB_SIZES = [128] * 24 + [64]
KS = [128, 128]                  # contraction chunks for K=256
GROUPS = [list(range(g * 3, min(g * 3 + 3, NB))) for g in range(9)]
ACT_E1_GROUPS = frozenset((4, 8))  # e1 exp groups on ACT (rest DVE); keeps
# ACT (~112us busy) and DVE (~130us) balanced so scores don't stall on the
# DVE queue late in each iteration

A16 = 128.0 / math.log(2.0)          # scale folded into wq: scores = A16*S
B_SCH = 16256.0 - 896.0 - 7.0        # Schraudolph bias incl. 2^-7 and c=7
ACT_SCALE = math.log(2.0) / 128.0    # undoes A16 on the exact-exp path
LN2_7 = -7.0 * math.log(2.0)         # exp bias; cancels in the divide

_GRAPH = None
NUM_DEVICES = 8
DEBUG = False


def _build_graph():
    import concourse.bass as bass  # noqa: F401
    import concourse.mybir as mybir
    import concourse.tile as tile
    from concourse import bacc
    from contextlib import ExitStack

    f32 = mybir.dt.float32
    bf16 = mybir.dt.bfloat16
    i16 = mybir.dt.int16
    Exp = mybir.ActivationFunctionType.Exp

    nc = bacc.Bacc("TRN2", target_bir_lowering=False, debug=False,
                   num_devices=NUM_DEVICES)
    xa_d = nc.dram_tensor("xa", [256, N], bf16, kind="ExternalInput").ap()
    xq_d = nc.dram_tensor("xq", [256, NCHUNK], bf16, kind="ExternalInput").ap()
    wq_d = nc.dram_tensor("wq", [256, 128], bf16, kind="ExternalInput").ap()
    wk_d = nc.dram_tensor("wk", [256, 128], bf16, kind="ExternalInput").ap()
    wv_d = nc.dram_tensor("wv", [256, 512], bf16, kind="ExternalInput").ap()
    qb_d = nc.dram_tensor("qb", [128, 1], f32, kind="ExternalInput").ap()
    pb_d = nc.dram_tensor("pb", [128, 2], f32, kind="ExternalInput").ap()
    wp_d = nc.dram_tensor("wp", [64, 8, 256], bf16, kind="ExternalInput").ap()
    out_d = nc.dram_tensor("out", [256, NCHUNK], f32, kind="ExternalOutput").ap()
    if DEBUG:
        rsr_d = nc.dram_tensor("rsr", [16, NSUB], f32,
                               kind="ExternalOutput").ap()
    else:
        rsr_d = nc.dram_tensor("rsr", [16, NSUB], f32).ap()
    if DEBUG:
        dbg = {nm: nc.dram_tensor("dbg_" + nm, shp, dt,
                                  kind="ExternalOutput").ap()
               for nm, shp, dt in [
                   ("qsb", [128, NCHUNK], bf16), ("ksb", [128, N], bf16),
                   ("klo", [128, N], bf16), ("khi2", [128, N], bf16),
                   ("vt", [128, NB * 2 * 4 * 65], bf16),
                   ("of", [64, 8 * NCHUNK], bf16)]}

    with tile.TileContext(nc) as tc, ExitStack() as stk:
        const = stk.enter_context(tc.tile_pool(name="const", bufs=1))
        xq_sb = const.tile([128, 2, NCHUNK], bf16, tag="xq")
        wq_sb = const.tile([128, 2, 128], bf16, tag="wq")
        wk_sb = const.tile([128, 2, 128], bf16, tag="wk")
        wv_sb = const.tile([128, 2, 512], bf16, tag="wv")
        wp_sb = const.tile([64, 8, 256], bf16, tag="wp")
        qb_sb = const.tile([128, 1], f32, tag="qb")
        pb_sb = const.tile([128, 2], f32, tag="pb")
        eb_sb = const.tile([128, 1], f32, tag="eb")
        k_lo = const.tile([128, N], bf16, tag="klo")
        k_hi = const.tile([128, N], bf16, tag="khi")
        q_lo = const.tile([128, NCHUNK], bf16, tag="qlo")
        q_hi = const.tile([128, NCHUNK], bf16, tag="qhi")
        # replicas shifted by +32 partitions so consecutive blocks of one head
        # use different PE row tiles
        k_lo2 = const.tile([128, N], bf16, tag="klo2")
        k_hi2 = const.tile([128, N], bf16, tag="khi2")
        q_lo2 = const.tile([128, NCHUNK], bf16, tag="qlo2")
        q_hi2 = const.tile([128, NCHUNK], bf16, tag="qhi2")
        # v'^T: [m-in-block, block, half, head-in-half, 64 v cols + ones col]
        vT_sb = const.tile([128, NB, 2, 4, 65], bf16, tag="vt")
        of_sb = const.tile([64, 8, NCHUNK], bf16, tag="of")
        y_sb = const.tile([128, 2, NCHUNK], f32, tag="y")
        xa_sb = const.tile([128, 2, N], bf16, tag="xa")
        k_sb = const.tile([128, N], bf16, tag="ksb")
        q_sb = const.tile([128, NCHUNK], bf16, tag="qsb")

        # ---- input DMAs, chunked across sync/gpsimd/scalar HWDGE queues so
        # the big loads stripe over many DMA rings instead of serializing.
        # xa n-quarter j feeds k-proj pair j and v' blocks ~6j..6j+6, so xa
        # chunks lead each queue in j order; weights slot between them ----
        NA8 = N // 8

        def xa_chunk(eng, j, kc):
            eng.dma_start(
                out=xa_sb[:, kc, j * NA8:(j + 1) * NA8],
                in_=xa_d[128 * kc:128 * kc + 128, j * NA8:(j + 1) * NA8])

        def xq_chunk(eng, j, kc):
            eng.dma_start(
                out=xq_sb[:, kc, j * NSUB:(j + 1) * NSUB],
                in_=xq_d[128 * kc:128 * kc + 128, j * NSUB:(j + 1) * NSUB])

        # n-chunk j of xa feeds k-proj chunk ~j/2 and v' blocks ~3j..3j+3;
        # the first four chunks land on four separate rings so the k/v
        # projections start ~6us in
        xa_chunk(nc.sync, 0, 0)
        xa_chunk(nc.scalar, 0, 1)
        xa_chunk(nc.gpsimd, 1, 0)
        xa_chunk(nc.gpsimd, 1, 1)
        nc.sync.dma_start(out=wq_sb[:, 0, :], in_=wq_d[0:128, :])
        nc.sync.dma_start(out=wq_sb[:, 1, :], in_=wq_d[128:256, :])
        nc.sync.dma_start(out=qb_sb[:], in_=qb_d)
        nc.scalar.dma_start(out=wv_sb[:, 0, :], in_=wv_d[0:128, :])
        nc.scalar.dma_start(out=wv_sb[:, 1, :], in_=wv_d[128:256, :])
        nc.gpsimd.dma_start(out=wk_sb[:, 0, :], in_=wk_d[0:128, :])
        nc.gpsimd.dma_start(out=wk_sb[:, 1, :], in_=wk_d[128:256, :])
        xa_chunk(nc.sync, 2, 0)
        xa_chunk(nc.scalar, 2, 1)
        xq_chunk(nc.gpsimd, 0, 0)
        xq_chunk(nc.gpsimd, 0, 1)
        xa_chunk(nc.sync, 3, 0)
        xa_chunk(nc.scalar, 3, 1)
        xa_chunk(nc.gpsimd, 4, 0)
        xa_chunk(nc.gpsimd, 4, 1)
        xq_chunk(nc.sync, 1, 0)
        xq_chunk(nc.scalar, 1, 1)
        xa_chunk(nc.sync, 5, 0)
        xa_chunk(nc.scalar, 5, 1)
        xa_chunk(nc.gpsimd, 6, 0)
        xa_chunk(nc.gpsimd, 6, 1)
        xa_chunk(nc.sync, 7, 0)
        xa_chunk(nc.scalar, 7, 1)
        nc.sync.dma_start(out=pb_sb[:], in_=pb_d)
        nc.sync.dma_start(out=wp_sb[:, 0:4, :], in_=wp_d[:, 0:4, :])
        nc.sync.dma_start(out=wp_sb[:, 4:8, :], in_=wp_d[:, 4:8, :])
        nc.vector.memset(eb_sb[:], LN2_7)
        nc.vector.memset(vT_sb[:, :, :, :, 64:65], 1.0)
        # zero-fill: K=32 score matmuls contract 16 zero rows per tile so the
        # PE array's unused rows hold zero weights (K=16 would leave stale
        # weights toggling against real data -> measurably more HAM throttle)
        for t in (k_lo, k_hi, k_lo2, k_hi2, q_lo, q_hi, q_lo2, q_hi2):
            nc.vector.memset(t[:], 0.0)

        with tc.tile_pool(name="pP", bufs=30) as pP, \
             tc.tile_pool(name="pEp", bufs=4) as pEp, \
             tc.tile_pool(name="psO", bufs=2, space="PSUM") as psO, \
             tc.tile_pool(name="psS", bufs=2, space="PSUM") as psS:

            # ---- projections: q then k (PE), kc-chunks interleaved across
            # bank pairs to hide the PSUM read-modify-write stall ----
            q_ps2 = [psO.tile([128, 512], f32, tag="ops", name=f"qps{c2}")
                     for c2 in range(2)]
            for kc in range(2):
                for c2 in range(2):
                    nc.tensor.matmul(
                        q_ps2[c2][0:128, 0:NSUB],
                        wq_sb[0:KS[kc], kc, :],
                        xq_sb[0:KS[kc], kc, c2 * NSUB:(c2 + 1) * NSUB],
                        start=(kc == 0), stop=(kc == 1))
            for c2 in range(2):
                nc.scalar.add(
                    q_sb[:, c2 * NSUB:(c2 + 1) * NSUB],
                    q_ps2[c2][0:128, 0:NSUB], qb_sb[:, 0:1])
            KP = [(512 * p, min(512, N - 512 * p)) for p in range(7)]
            for p0 in range(0, 7, 2):
                ps = [(p, psO.tile([128, 512], f32, tag="ops", name=f"kps{p}"))
                      for p in range(p0, min(p0 + 2, 7))]
                for kc in range(2):
                    for p, k_ps in ps:
                        c0, cw = KP[p]
                        nc.tensor.matmul(
                            k_ps[0:128, 0:cw],
                            wk_sb[0:KS[kc], kc, :],
                            xa_sb[0:KS[kc], kc, c0:c0 + cw],
                            start=(kc == 0), stop=(kc == 1))
                for p, k_ps in ps:
                    c0, cw = KP[p]
                    if p % 2 == 0:
                        nc.vector.tensor_copy(k_sb[:, c0:c0 + cw],
                                              k_ps[0:128, 0:cw])
                    else:
                        nc.scalar.copy(k_sb[:, c0:c0 + cw], k_ps[0:128, 0:cw])

            # ---- q/k regroup: heads are already at 32-aligned slots in
            # q_sb/k_sb (host-side weight permutation), so each of q_lo /
            # q_hi / replicas fills with one strided DMA (two for the
            # wrapped +32 replica) ----
            def regroup(dst_lo, dst_hi, dst_lo2, dst_hi2, src, n, engs):
                for h in range(8):
                    dt_ = dst_lo if h < 4 else dst_hi
                    dt2 = dst_lo2 if h < 4 else dst_hi2
                    bp_ = 32 * (h % 4)
                    bp2 = (bp_ + 32) % 128
                    eng = engs[h % len(engs)]
                    eng.dma_start(out=dt_[bp_:bp_ + 16, 0:n],
                                  in_=src[16 * h:16 * h + 16, 0:n])
                    eng.dma_start(out=dt2[bp2:bp2 + 16, 0:n],
                                  in_=src[16 * h:16 * h + 16, 0:n])

            regroup(q_lo, q_hi, q_lo2, q_hi2, q_sb, NCHUNK, [nc.gpsimd])
            regroup(k_lo, k_hi, k_lo2, k_hi2, k_sb, N, [nc.sync, nc.gpsimd])

            # ---- main attention loop, software-pipelined ----
            # iteration i = (head-pair, n-chunk). During iteration i's
            # scores+exp phase the PE runs iteration i-1's o'-accumulation
            # matmuls as filler (iterations 0/1 run the v'^T projection, and
            # iteration 5 additionally chunk 0's output projection).
            # PSUM: scores 2 x 3 banks + o'/v'/wp pool 2 x 1 bank = 8 banks.
            PAIRS = [(0, 2), (1, 3), (4, 6), (5, 7)]
            ITERS = [(pair, c2) for c2 in range(2) for pair in PAIRS]

            def make_o_filler(pair, e, p_tile, i, mb, o_ps2):
                def emit():
                    h = pair[e]
                    pbi = MB_SIZES[mb]
                    nc.tensor.matmul(
                        o_ps2[e][0:65, 0:NSUB],
                        vT_sb[0:pbi, mb, h // 4, h % 4, :],
                        p_tile[0:pbi, i, 0:NSUB],
                        start=(mb == 0), stop=(mb == NB - 1))
                return emit

            def make_v_filler(mb):
                def emit():
                    pb_ = MB_SIZES[mb]
                    vt_ps = psO.tile([128, 2, 4, 64], f32, tag="ops",
                                     name=f"vtps{mb}")
                    for kc in range(2):
                        nc.tensor.matmul(
                            vt_ps[0:pb_, :, :, :],
                            xa_sb[0:KS[kc], kc, mb * 128:mb * 128 + pb_],
                            wv_sb[0:KS[kc], kc, :],
                            start=(kc == 0), stop=(kc == 1))
                    if mb % 2 == 0:
                        nc.vector.tensor_copy(
                            vT_sb[0:pb_, mb, :, :, 0:64], vt_ps[0:pb_, :, :, :])
                    else:
                        nc.scalar.copy(
                            vT_sb[0:pb_, mb, :, :, 0:64], vt_ps[0:pb_, :, :, :])
                return emit

            def make_wp_jobs(c2):
                # contract heads in pipeline-completion order: the last
                # pair's heads (5, 7) come last so earlier matmuls run while
                # the final epilogue's divide chain is still in flight
                # contract heads in pipeline-completion order; the flush
                # processes e1 (head 7) before e0 (head 5), so chunk 1's
                # contraction ends on head 5
                nc0 = c2 * NSUB
                KC_ORDER = (0, 2, 1, 3, 4, 6, 5, 7) if c2 == 0 else \
                           (0, 2, 1, 3, 4, 6, 7, 5)

                def job(mo, pool, tag):
                    def emit():
                        y_ps = pool.tile([128, 512], f32, tag=tag,
                                         name=f"yps{c2}{mo}",
                                         padded_shape=None)
                        for j, kc in enumerate(KC_ORDER):
                            nc.tensor.matmul(
                                y_ps[0:128, 0:NSUB],
                                wp_sb[0:64, kc, mo * 128:(mo + 1) * 128],
                                of_sb[0:64, kc, nc0:nc0 + NSUB],
                                start=(j == 0), stop=(j == 7))
                        nc.vector.tensor_scalar_add(
                            y_sb[:, mo, nc0:nc0 + NSUB], y_ps[0:128, 0:NSUB],
                            pb_sb[:, mo:mo + 1])
                        nc.sync.dma_start(
                            out=out_d[mo * 128:(mo + 1) * 128,
                                      nc0:nc0 + NSUB],
                            in_=y_sb[:, mo, nc0:nc0 + NSUB])
                    return emit
                if c2 == 0:
                    return [job(0, psO, "ops"), job(1, psO, "ops")]
                return [job(0, psS, "sps"), job(1, psS, "sps")]

            def divide_chain(pair, c2, o_ps2, it):
                # evacuate o' PSUM promptly (frees the banks for the next
                # iteration's fillers; the divide chain below can lag); den
                # row moves to partitions 0-1 via SBUF->SBUF DMA for the
                # custom-DVE reciprocal, then DRAM bounce for the broadcast.
                # Chain DMAs ride the quiet gpsimd queue, not sync.
                nc0 = c2 * NSUB
                rsh = pEp.tile([66, 2, NSUB], f32, tag="rsh", bufs=2,
                               name=f"rsh{it}")
                nc.scalar.copy(rsh[64:65, 0, :], o_ps2[0][64:65, 0:NSUB])
                nc.vector.tensor_copy(rsh[64:65, 1, :], o_ps2[1][64:65, 0:NSUB])
                ous = []
                for e in range(2):
                    ou = pEp.tile([64, NSUB], f32, tag=f"ou{e}",
                                  name=f"ou{it}{e}", bufs=2)
                    nc.scalar.copy(ou[0:64, :], o_ps2[e][0:64, 0:NSUB])
                    ous.append(ou)
                rb2 = pEp.tile([2, NSUB], f32, tag="rb2", bufs=2,
                               name=f"rb2{it}")
                nc.gpsimd.dma_start(out=rb2[0:2, :], in_=rsh[64:65, :, :])
                rbr = pEp.tile([2, NSUB], f32, tag="rbr", bufs=2,
                               name=f"rbr{it}")
                scr = pEp.tile([2, NSUB], f32, tag="scr", bufs=2,
                               name=f"scr{it}")
                nc.vector.reciprocal_approx_accurate(
                    out=rbr[0:2, :], in_=rb2[0:2, :], scratch=scr[0:2, :])
                nc.gpsimd.dma_start(out=rsr_d[2 * it:2 * it + 2, :],
                                    in_=rbr[0:2, :])
                for e in range(2):
                    h = pair[e]
                    rbc = pEp.tile([64, NSUB], f32, tag=f"rbc{e}", bufs=2,
                                   name=f"rbc{it}{e}")
                    nc.gpsimd.dma_start(
                        out=rbc[0:64, :],
                        in_=rsr_d[2 * it + e:2 * it + e + 1,
                                  :].partition_broadcast(64))
                    # multiply on Pool (SBUF-only operands): keeps the DVE
                    # queue clear of mid-iteration latency spikes
                    nc.gpsimd.tensor_mul(
                        out=of_sb[0:64, h, nc0:nc0 + NSUB],
                        in0=ous[e][0:64, :], in1=rbc[0:64, :])

            prev = None  # (pair, c2, p_tiles, it) of the previous iteration
            for it in range(len(ITERS) + 1):
                cur = ITERS[it] if it < len(ITERS) else None
                fillers = []
                if it == 0:
                    # v' blocks 0-14 run immediately (they only need xa/wv,
                    # ready early) while the first scores wait on the q/k
                    # regroup; 15-24 interleave with iteration 0's groups
                    for mb in range(15):
                        make_v_filler(mb)()
                    fillers = [make_v_filler(mb) for mb in range(15, NB)]
                if prev is not None:
                    ppair, pc2, p_tiles, pit = prev
                    o_ps2 = [psO.tile([128, 512], f32, tag="ops",
                                      name=f"ops{e}") for e in range(2)]
                    for g2, blocks2 in enumerate(GROUPS):
                        for i2, mb2 in enumerate(blocks2):
                            for e in range(2):
                                fillers.append(make_o_filler(
                                    ppair, e, p_tiles[g2][e], i2, mb2, o_ps2))
                if it == 5:
                    fillers.extend(make_wp_jobs(0))
                if cur is None:
                    # flush: run each head's o' fillers then its divide chain
                    # immediately, so head 0's chain hides under head 1's 25
                    # filler matmuls
                    # e1 first: its divide chain (the longest exposed tail)
                    # hides under e0's 25 filler matmuls
                    nc0f = pc2 * NSUB
                    for e in (1, 0):
                        for job in fillers[e::2]:
                            job()
                        # flush: shortest chain — den out, 64-row broadcast,
                        # reciprocal at partitions 0-63, multiply straight
                        # from the o' PSUM (no bank reuse pressure at flush)
                        h = ppair[e]
                        rshF = pEp.tile([66, 2, NSUB], f32, tag="rsh",
                                        bufs=2, name=f"rshF{e}")
                        if e == 0:
                            nc.scalar.copy(rshF[64:65, 0, :],
                                           o_ps2[e][64:65, 0:NSUB])
                        else:
                            nc.vector.tensor_copy(rshF[64:65, 0, :],
                                                  o_ps2[e][64:65, 0:NSUB])
                        nc.gpsimd.dma_start(
                            out=rsr_d[2 * pit + e:2 * pit + e + 1, :],
                            in_=rshF[64:65, 0, :])
                        rbcF = pEp.tile([64, NSUB], f32, tag=f"rbc{e}",
                                        bufs=2, name=f"rbcF{e}")
                        nc.gpsimd.dma_start(
                            out=rbcF[0:64, :],
                            in_=rsr_d[2 * pit + e:2 * pit + e + 1,
                                      :].partition_broadcast(64))
                        rbrF = pEp.tile([64, NSUB], f32, tag=f"rbrF{e}",
                                        bufs=1, name=f"rbrF{e}")
                        scrF = pEp.tile([64, NSUB], f32, tag=f"scrF{e}",
                                        bufs=1, name=f"scrF{e}")
                        nc.vector.reciprocal_approx_accurate(
                            out=rbrF[0:64, :], in_=rbcF[0:64, :],
                            scratch=scrF[0:64, :])
                        nc.vector.tensor_mul(
                            out=of_sb[0:64, h, nc0f:nc0f + NSUB],
                            in0=o_ps2[e][0:64, 0:NSUB], in1=rbrF[0:64, :])
                    for job in make_wp_jobs(1):
                        job()
                    if DEBUG:
                        nc.sync.dma_start(out=dbg["qsb"], in_=q_sb[:])
                        nc.sync.dma_start(out=dbg["ksb"], in_=k_sb[:])
                        nc.sync.dma_start(out=dbg["klo"], in_=k_lo[:])
                        nc.sync.dma_start(out=dbg["khi2"], in_=k_hi2[:])
                        nc.sync.dma_start(out=dbg["vt"], in_=vT_sb[:])
                        nc.sync.dma_start(out=dbg["of"], in_=of_sb[:])
                    break
                pair, c2 = cur
                kts = [k_lo if h < 4 else k_hi for h in pair]
                qts = [q_lo if h < 4 else q_hi for h in pair]
                kts2 = [k_lo2 if h < 4 else k_hi2 for h in pair]
                qts2 = [q_lo2 if h < 4 else q_hi2 for h in pair]
                bps = [32 * (h % 4) for h in pair]
                bps2 = [(32 * (h % 4) + 32) % 128 for h in pair]
                p_tiles = []
                nfill = len(fillers)
                nc0 = c2 * NSUB
                for g, blocks in enumerate(GROUPS):
                    gsz = len(blocks)
                    pb_ = MB_SIZES[blocks[-1]]
                    # two decoupled (PSUM-buffer, exp-engine) chains; score
                    # matmuls interleave (mb, e) across 4 rotating K=16 row
                    # tiles for PE stream concurrency
                    s_ps2 = [psS.tile([128, 3, 512], f32, tag="sps",
                                      name=f"sps{e}") for e in range(2)]
                    for i, mb in enumerate(blocks):
                        pbi = MB_SIZES[mb]
                        for e in range(2):
                            if mb % 2 == 0:
                                kte, qte, be = kts[e], qts[e], bps[e]
                            else:
                                kte, qte, be = kts2[e], qts2[e], bps2[e]
                            nc.tensor.matmul(
                                s_ps2[e][0:pbi, i, 0:NSUB],
                                kte[be:be + 32, mb * 128:mb * 128 + pbi],
                                qte[be:be + 32, nc0:nc0 + NSUB],
                                start=True, stop=True,
                                tile_position=(be, 0))
                    p_sb2 = []
                    for e in range(2):
                        p_sb = pP.tile([128, 3, NSUB], bf16, tag="psb",
                                       name=f"psb{e}")
                        if e == 0 or g in ACT_E1_GROUPS:
                            nc.scalar.activation(
                                out=p_sb[0:pb_, 0:gsz, 0:NSUB],
                                in_=s_ps2[e][0:pb_, 0:gsz, 0:NSUB], func=Exp,
                                bias=eb_sb[0:pb_, 0:1], scale=ACT_SCALE)
                        else:
                            nc.vector.tensor_scalar_add(
                                p_sb[0:pb_, 0:gsz, 0:NSUB].bitcast(i16),
                                s_ps2[e][0:pb_, 0:gsz, 0:NSUB], B_SCH)
                        p_sb2.append(p_sb)
                    p_tiles.append(p_sb2)
                    # interleave previous iteration's o' matmuls as PE
                    # filler in three big batches (after groups 2/4/6) —
                    # fewer tile-config switches than per-group batches; the
                    # epilogue at slot 6 still frees the o' banks in time
                    FS = {2: (0, 1), 4: (1, 2), 6: (2, 3)}
                    if g in FS:
                        lo = nfill * FS[g][0] // 3
                        hi = nfill * FS[g][1] // 3
                        for job in fillers[lo:hi]:
                            job()
                    if g == 6 and prev is not None:
                        divide_chain(ppair, pc2, o_ps2, pit)
                prev = (pair, c2, p_tiles, it)

    nc.compile()
    return nc


def get_graph():
    global _GRAPH
    if _GRAPH is None:
        _GRAPH = _build_graph()
    return _GRAPH


def make_in_maps(x, wq, sq, bq, wk, sk, bk, wv, sv, bv, wp, sp, bp):
    import ml_dtypes
    bf = ml_dtypes.bfloat16
    f = np.float32
    x2 = np.asarray(x, f).reshape(B, C, N)
    wq = np.asarray(wq, f); sq = np.asarray(sq, f); bq = np.asarray(bq, f)
    wk = np.asarray(wk, f); sk = np.asarray(sk, f)
    wv = np.asarray(wv, f); sv = np.asarray(sv, f); bv = np.asarray(bv, f)
    wp = np.asarray(wp, f); sp = np.asarray(sp, f); bp = np.asarray(bp, f)

    wq_eff = ((wq * sq[:, None]).T * A16).astype(f)   # (256, 128), pre-scaled
    wk_eff = (wk * sk[:, None]).T.astype(f)           # k-bias dropped
    qb_eff = (bq * A16).astype(f)
    wv_base = wv * sv[:, None]  # (512, 256)
    wv_arr = np.zeros((256, 512), f)
    for h in range(NUM_HEADS):
        col = 256 * (h // 4) + 64 * (h % 4)
        wv_arr[:, col:col + 64] = wv_base[64 * h:64 * h + 64, :].T
    wp_sc = wp * sp[:, None]                 # (256, 512)
    wp_eff = wp_sc.T.astype(f)               # (512, 256), row c = 64h+d
    wp_arr = wp_eff.reshape(8, 64, 256).transpose(1, 0, 2).copy()
    pb_fold = (bp + wp_sc @ bv).astype(f)    # v-bias folded into out-bias
    pb_arr = pb_fold.reshape(2, 128).T.copy()  # pb_arr[d, mo] = pb'[128*mo+d]
    in_maps = []
    for core in range(8):
        b, j = core // 4, core % 4
        xa_full = np.ascontiguousarray(x2[b])
        xq_c = np.ascontiguousarray(xa_full[:, j * NCHUNK:(j + 1) * NCHUNK])
        in_maps.append(dict(
            xa=xa_full.astype(bf), xq=xq_c.astype(bf),
            wq=wq_eff.astype(bf), wk=wk_eff.astype(bf),
            wv=wv_arr.astype(bf), wp=wp_arr.astype(bf),
            qb=qb_eff.reshape(128, 1).astype(f),
            pb=pb_arr.astype(f)))
    return in_maps


def assemble_output(results):
    y = np.zeros((B, C, N), np.float32)
    for core in range(8):
        b, j = core // 4, core % 4
        y[b, :, j * NCHUNK:(j + 1) * NCHUNK] = results[core]["out"]
    return y.reshape(B, C, HH, WW)


def kernel(**inputs):
    from concourse.bass_utils import run_bass_kernel_spmd
    nc = get_graph()
    in_maps = make_in_maps(**inputs)
    res = run_bass_kernel_spmd(nc, in_maps, core_ids=list(range(8)))
    return assemble_output(res.results)


if __name__ == "__main__":
    rng = np.random.default_rng(0)
    ins = dict(
        x=rng.standard_normal((2, 256, 56, 56), np.float32).astype(np.float32),
        wq=(rng.standard_normal((128, 256)) * 0.05).astype(np.float32),
        sq=rng.random(128).astype(np.float32),
        bq=(rng.standard_normal(128) * 0.05).astype(np.float32),
        wk=(rng.standard_normal((128, 256)) * 0.05).astype(np.float32),
        sk=rng.random(128).astype(np.float32),
        bk=(rng.standard_normal(128) * 0.05).astype(np.float32),
        wv=(rng.standard_normal((512, 256)) * 0.05).astype(np.float32),
        sv=rng.random(512).astype(np.float32),
        bv=(rng.standard_normal(512) * 0.05).astype(np.float32),
        wp=(rng.standard_normal((256, 512)) * 0.05).astype(np.float32),
        sp=rng.random(256).astype(np.float32),
        bp=(rng.standard_normal(256) * 0.05).astype(np.float32),
    )
    out = kernel(**ins)
    print("out", out.shape, out.dtype, float(np.abs(out).mean()))
